# revision 30
# baseline (speedup 1.0000x reference)
"""Trainium2 Bass kernel for nn_Downsample_Spa: sigma-conv + gaussian unfold downsample.

Math (per batch image, one NeuronCore each; batch of 8 -> 8 cores):
  xp = reflect_pad(x)                                  # [64,130,130]
  sigma[o,p] = clamp(BN(conv3x3(xp))[o,p], 1e-4)       # at stride-2 positions p only
  graw[o,p]  = exp(-0.5*d2[o]/sigma^2 - ln64) / sigma  # /64 guards fp16 range; cancels in the ratio
  out[c,p]   = sum_o graw[o,p]*xp[c,p+off(o)] / sum_o graw[o,p]

Design (v6):
 - partitions = (row-half hh, channel c) = 128; host pre-pads (reflect), fp16, and
   parity-splits columns into TWO planes (w=2j / w=2j+1); taps at w=2j+2 read
   plane0 at col+1 (still step-1). Input 2.2MB/core.
 - conv: 9 accumulating fp16 matmuls per 512-position block, block-diagonal
   weights (M=18 = both row halves). All 4 conv blocks run contiguously up front
   (dense PE stretch keeps the HAM clock-gate open); sigma f32 in PSUM.
 - g pipeline per half [18,1024]: clamp via tensor_scalar, custom-DVE fast
   reciprocal, ACT Square+Exp (per-partition scale/bias), mult -> gb bf16.
 - unfold per block: one-hot bf16 matmuls broadcast gb to the 128 partitions
   (3-tap PSUM rowgroups + Srep sum); ACT copies to one fp16 gc tile; 9 fp16
   products into two 4-slot tiles -> one 2x-mode t4 add, then t2/t1/+center,
   *1/Srep; tail offloaded to gpsimd on early blocks; fp16 out DMA per block.
 - warm-up matmuls on a memset scratch tile (no DMA dependency); inputs DMA'd
   over both hw queues (sync+scalar) so chunk 0 lands in ~2us.
"""

import os
import sys

import numpy as np

if "/opt/trn_rl_repo" not in sys.path:
    sys.path.insert(0, "/opt/trn_rl_repo")

K = 3
BN_EPS = 1e-5
SIGMA_MIN = 1e-4
GSCALE_LN = float(np.log(64.0))   # graw scaled by 1/64 (folded into exp bias)
N, C, H, W = 8, 64, 128, 128
HO = WO = 64
HH = 2
RS = 65                  # padded-row slots per partition-half
HOC = 32
NBLK = 4
BR = HOC // NBLK         # 8 output rows per block
NPOS = BR * WO           # 512
NP2 = 2 * NPOS
PL = 2                   # x col-parity planes: w=2j / w=2j+1
JW = 66                  # j slots per plane (65 used, 66 for alignment)
CR = 17                  # rows per DMA chunk tile (16 + 1 overlap)

# f32 consts tensor columns
_D2 = 0                  # -0.5*d2[o] per (hh,o)
_BC = 1                  # bn_bias - sigma_min
_LB = 2                  # exp bias: constant -ln(64) per partition
_EPS = 3                 # sigma clamp floor
_NCC = 4

_STATE = {}


def _build_consts(conv_w, bn_gamma, bn_beta, bn_mean, bn_var):
    s = (bn_gamma / np.sqrt(bn_var + BN_EPS)).astype(np.float32)
    wf = conv_w.astype(np.float32) * s[:, None, None, None]           # [9,64,3,3]
    bias = (bn_beta - bn_mean * s).astype(np.float32)

    cst = np.zeros((18, _NCC), np.float32)
    d2 = np.array([(kk // 3 - 1) ** 2 + (kk % 3 - 1) ** 2 for kk in range(9)], np.float32)
    for hh in range(HH):
        cst[hh * 9:hh * 9 + 9, _D2] = -0.5 * d2
        cst[hh * 9:hh * 9 + 9, _BC] = bias - SIGMA_MIN
        cst[hh * 9:hh * 9 + 9, _LB] = -GSCALE_LN
        cst[hh * 9:hh * 9 + 9, _EPS] = SIGMA_MIN

    # conv weights, block-diagonal per tap: win[k=hh*64+c, tap*18 + hh*9+o]
    win = np.zeros((128, 9 * 18), np.float16)
    for tap in range(9):
        i, j = tap // 3, tap % 3
        for hh in range(HH):
            win[hh * 64:hh * 64 + 64, tap * 18 + hh * 9:tap * 18 + hh * 9 + 9] = \
                wf[:, :, i, j].T.astype(np.float16)

    # one-hot / ones broadcast weights: gin[k=hh*9+o, tap*128 + hh*64+c]
    import ml_dtypes
    gin = np.zeros((18, 10 * 128), ml_dtypes.bfloat16)
    for hh in range(HH):
        gin[hh * 9:hh * 9 + 9, 9 * 128 + hh * 64:9 * 128 + hh * 64 + 64] = 1.0
        for tap in range(9):
            gin[hh * 9 + tap, tap * 128 + hh * 64:tap * 128 + hh * 64 + 64] = 1.0
    return cst, win, gin


def _build_bass(for_sim=False):
    import concourse.bass as bass
    import concourse.tile as tile
    from concourse import mybir

    f32 = mybir.dt.float32
    f16 = mybir.dt.float16
    bf16 = mybir.dt.bfloat16
    MULT = mybir.AluOpType.mult
    ADD = mybir.AluOpType.add
    MAX = mybir.AluOpType.max
    AF = mybir.ActivationFunctionType

    if for_sim:
        nc = bass.Bass("TRN2", target_bir_lowering=False, detect_race_conditions=False)
    else:
        from concourse import bacc
        nc = bacc.Bacc()
    xin = nc.dram_tensor("xin", [128, RS, PL, JW], f16, kind="ExternalInput")
    cin = nc.dram_tensor("cin", [18, _NCC], f32, kind="ExternalInput")
    win = nc.dram_tensor("win", [128, 9 * 18], f16, kind="ExternalInput")
    gin = nc.dram_tensor("gin", [18, 10 * 128], bf16, kind="ExternalInput")
    out = nc.dram_tensor("out", [128, HOC, WO], f16, kind="ExternalOutput")

    with tile.TileContext(nc) as tc:
        from contextlib import ExitStack
        with ExitStack() as ctx:
            big = ctx.enter_context(tc.tile_pool(name="big", bufs=1))
            gsb = ctx.enter_context(tc.tile_pool(name="gsb", bufs=3))
            y_p = ctx.enter_context(tc.tile_pool(name="y", bufs=3))
            ps_s = ctx.enter_context(tc.tile_pool(name="ps_s", bufs=2, space="PSUM"))
            ps_g = ctx.enter_context(tc.tile_pool(name="ps_g", bufs=2, space="PSUM"))

            # warm-up operand: memset scratch, so the PE can start before any DMA
            scr = big.tile([128, 162], f16)
            nc.vector.memset(scr[:], 0.0)

            # ---- input DMAs over both hw queues; chunk 0 first ----
            ws = big.tile([128, 9 * 18], f16)
            cs = big.tile([18, _NCC], f32)
            gs = big.tile([18, 10 * 128], bf16)
            # 4 input chunks FIFO on the sync hw queue: chunk 0 gets the full
            # queue rate and lands first; tiny consts ride the scalar queue
            xsk = []
            for b in range(NBLK):
                xs = big.tile([128, CR, PL, JW], f16, tag=f"xs{b}")
                xsk.append(xs)
            nc.scalar.dma_start(out=ws[:], in_=win[:])
            nc.sync.dma_start(out=xsk[0][:], in_=xin[:, 0:CR, :, :])
            nc.scalar.dma_start(out=cs[:], in_=cin[:])
            nc.scalar.dma_start(out=gs[:], in_=gin[:])
            nc.sync.dma_start(out=xsk[1][:], in_=xin[:, 16:16 + CR, :, :])
            nc.sync.dma_start(out=xsk[2][:], in_=xin[:, 32:32 + CR, :, :])
            nc.sync.dma_start(out=xsk[3][:], in_=xin[:, 48:48 + CR, :, :])

            def xtap(tap, blk):
                # [128, 8, 64] fp16 step-1 view for tap (i,j) in block blk
                i, j = tap // 3, tap % 3
                if j < 2:
                    return xsk[blk][:, i:i + 2 * BR - 1:2, j, 0:WO]
                return xsk[blk][:, i:i + 2 * BR - 1:2, 0, 1:WO + 1]

            def xpair(i, blk):
                # [128, 2, 8, 64] planes (w=2j, 2j+1) for tap row i
                return xsk[blk][:, i:i + 2 * BR - 1:2, 0:PL, 0:WO].transpose([0, 2, 1, 3])

            # ---- PE warm-up on scratch (~3.4us) so HAM un-throttles pre-conv ----
            wu = ps_s.tile([18, NPOS], f32, tag="sig")
            for _ in range(30):
                nc.tensor.matmul(wu[:, 0:162], scr[:, 0:18], scr[:, 0:162],
                                 start=True, stop=True)

            # ---- conv: per-block sigma [18,512] in PSUM, clamped into a
            # [18,1024] SBUF half tile ----
            sigh = []
            for h in range(2):
                sc = gsb.tile([18, NP2], f32, tag=f"sc{h}")
                for sub in range(2):
                    blk = 2 * h + sub
                    sig = ps_s.tile([18, NPOS], f32, tag="sig")
                    for tap in range(9):
                        nc.tensor.matmul(
                            sig[:],
                            ws[:, tap * 18:(tap + 1) * 18],
                            xtap(tap, blk),
                            start=(tap == 0), stop=(tap == 8),
                        )
                    # clamp on ACT, not DVE: max(v, f) == Relu(v - f) + f exactly
                    sp = gsb.tile([18, NPOS], f32, tag="sp")
                    nc.scalar.activation(out=sp[:], in_=sig[:], func=AF.Relu,
                                         bias=cs[:, _BC:_BC + 1])
                    nc.scalar.activation(out=sc[:, sub * NPOS:(sub + 1) * NPOS],
                                         in_=sp[:], func=AF.Identity,
                                         bias=cs[:, _EPS:_EPS + 1])
                sigh.append(sc)

            def g_emit(sc, lo, hi):
                npo = hi - lo
                inv = gsb.tile([18, NP2], f32, tag="inv")
                nc.vector.reciprocal_approx_fast(out=inv[:, 0:npo], in_=sc[:, lo:hi])
                qt = gsb.tile([18, NP2], f32, tag="qt")
                nc.scalar.activation(out=qt[:, 0:npo], in_=inv[:, 0:npo], func=AF.Square)
                et = gsb.tile([18, NP2], f32, tag="et")
                nc.scalar.activation(out=et[:, 0:npo], in_=qt[:, 0:npo], func=AF.Exp,
                                     scale=cs[:, _D2:_D2 + 1],
                                     bias=cs[:, _LB:_LB + 1])
                gb = gsb.tile([18, NP2], bf16, tag="gb")
                nc.vector.tensor_tensor(out=gb[:, 0:npo], in0=et[:, 0:npo],
                                        in1=inv[:, 0:npo], op=MULT)
                return gb

            def unfold_emit(blk, gbs, late):
                # gbs: [18, 512] slice (this block's g, taps on partitions)
                Srep = ps_s.tile([128, NPOS], f32, tag="sig")
                nc.tensor.matmul(Srep[:], gs[:, 9 * 128:10 * 128], gbs,
                                 start=True, stop=True)
                rr = y_p.tile([128, BR, WO], f32, tag="rr")
                nc.vector.reciprocal_approx_fast(out=rr[:], in_=Srep[:])

                gc = y_p.tile([128, 9, BR, WO], f16, tag="gc")
                for g in range(3):  # rowgroups: taps (0,1,2), (3,4,5), (6,7,8)
                    g3 = ps_g.tile([128, 3, NPOS], f32, tag="grep")
                    for k in range(3):
                        tap = 3 * g + k
                        nc.tensor.matmul(g3[:, k], gs[:, tap * 128:(tap + 1) * 128],
                                         gbs, start=True, stop=True)
                    nc.scalar.activation(out=gc[:, 3 * g:3 * g + 3], in_=g3[:],
                                         func=AF.Copy)

                # products, all fp16: ytA = [r0j0, r0j1, r0j2, r1j0],
                # ytB = [r2j0, r2j1, r2j2, r1j2], yC = center
                ytA = y_p.tile([128, 4, BR, WO], f16, tag="ytA")
                ytB = y_p.tile([128, 4, BR, WO], f16, tag="ytB")
                yC = y_p.tile([128, BR, WO], f16, tag="yC")
                nc.vector.tensor_tensor(out=ytA[:, 0:2], in0=xpair(0, blk),
                                        in1=gc[:, 0:2], op=MULT)
                nc.vector.tensor_tensor(out=ytA[:, 2], in0=xtap(2, blk),
                                        in1=gc[:, 2], op=MULT)
                nc.vector.tensor_tensor(out=ytA[:, 3], in0=xtap(3, blk),
                                        in1=gc[:, 3], op=MULT)
                nc.vector.tensor_tensor(out=yC[:], in0=xtap(4, blk),
                                        in1=gc[:, 4], op=MULT)
                nc.vector.tensor_tensor(out=ytB[:, 3], in0=xtap(5, blk),
                                        in1=gc[:, 5], op=MULT)
                nc.vector.tensor_tensor(out=ytB[:, 0:2], in0=xpair(2, blk),
                                        in1=gc[:, 6:8], op=MULT)
                nc.vector.tensor_tensor(out=ytB[:, 2], in0=xtap(8, blk),
                                        in1=gc[:, 8], op=MULT)

                # pair tree (fp16, 2x-mode contiguous tiles) + center + normalize
                t4 = y_p.tile([128, 4, BR, WO], f16, tag="t4")
                nc.vector.tensor_tensor(out=t4[:], in0=ytA[:], in1=ytB[:], op=ADD)
                eng1 = nc.vector if late else nc.gpsimd
                t2 = y_p.tile([128, 2, BR, WO], f16, tag="t2")
                eng1.tensor_tensor(out=t2[:], in0=t4[:, 0:2], in1=t4[:, 2:4], op=ADD)
                t1 = y_p.tile([128, BR, WO], f16, tag="t1")
                eng1.tensor_tensor(out=t1[:], in0=t2[:, 0], in1=t2[:, 1], op=ADD)
                tC = y_p.tile([128, BR, WO], f16, tag="tC")
                eng1.tensor_tensor(out=tC[:], in0=t1[:], in1=yC[:], op=ADD)
                o16 = y_p.tile([128, BR, WO], f16, tag="o16")
                eng1.tensor_tensor(out=o16[:], in0=tC[:], in1=rr[:], op=MULT)
                nc.sync.dma_start(out=out[:, BR * blk:BR * (blk + 1), :], in_=o16[:])

            gb0a = g_emit(sigh[0], 0, NPOS)        # block 0 alone: starts DVE early
            unfold_emit(0, gb0a[:, 0:NPOS], late=False)
            gb0b = g_emit(sigh[0], NPOS, NP2)
            unfold_emit(1, gb0b[:, 0:NPOS], late=False)
            gb1 = g_emit(sigh[1], 0, NP2)
            unfold_emit(2, gb1[:, 0:NPOS], late=False)
            unfold_emit(3, gb1[:, NPOS:NP2], late=True)

    if not for_sim and not nc.is_finalized():
        nc.finalize()
    return nc


def _prep_inputs(x, conv_w, bn_gamma, bn_beta, bn_mean, bn_var):
    cst, win, gin = _build_consts(conv_w, bn_gamma, bn_beta, bn_mean, bn_var)
    xp = np.pad(np.asarray(x, np.float32), ((0, 0), (0, 0), (1, 1), (1, 1)),
                mode="reflect").astype(np.float16)                    # [8,64,130,130]
    in_maps = []
    for n in range(N):
        xc = np.concatenate([xp[n, :, 0:RS, :], xp[n, :, 64:64 + RS, :]], axis=0)
        xpl = np.zeros((128, RS, PL, JW), np.float16)
        xpl[:, :, 0, 0:65] = xc[:, :, 0:130:2]
        xpl[:, :, 1, 0:65] = xc[:, :, 1:130:2]
        in_maps.append({"xin": xpl, "cin": cst, "win": win, "gin": gin})
    return in_maps


def _gather(results):
    out = np.empty((N, C, HO, WO), np.float32)
    for n in range(N):
        d = np.asarray(results[n]["out"], np.float32)
        out[n, :, 0:HOC, :] = d[0:64]
        out[n, :, HOC:, :] = d[64:128]
    return out


def _enable_axon_trace():
    """Register the NTFF profile hook that this image's antenv lacks."""
    if _STATE.get("trace_hooked"):
        return
    import types
    import antenv
    from concourse import bass_utils
    mod = types.ModuleType("antenv.axon_hooks")
    mod._hook = None
    mod.set_axon_ntff_profile_hook = lambda h: setattr(mod, "_hook", h)
    mod.get_axon_ntff_profile_hook = lambda: mod._hook
    sys.modules["antenv.axon_hooks"] = mod
    antenv.axon_hooks = mod
    from trn_agent_boot.trn_boot import _ntff_profile_via_ctypes
    mod._hook = _ntff_profile_via_ctypes("/opt/axon/libaxon_pjrt.so")
    bass_utils.upload_artifacts = lambda tmpdir: tmpdir
    _STATE["trace_hooked"] = True


def run(x, conv_w, bn_gamma, bn_beta, bn_mean, bn_var, trace=False):
    from concourse.bass_utils import run_bass_kernel_spmd
    if trace:
        _enable_axon_trace()
    if "nc" not in _STATE:
        _STATE["nc"] = _build_bass()
    in_maps = _prep_inputs(x, conv_w, bn_gamma, bn_beta, bn_mean, bn_var)
    res = run_bass_kernel_spmd(_STATE["nc"], in_maps, list(range(N)), trace=trace)
    _STATE["last"] = res
    return _gather(res.results)


def kernel(x, conv_w, bn_gamma, bn_beta, bn_mean, bn_var):
    return run(x, conv_w, bn_gamma, bn_beta, bn_mean, bn_var,
               trace=bool(int(os.environ.get("KERNEL_TRACE", "0"))))


# revision 31
# speedup vs baseline: 1.0757x; 1.0757x over previous
"""Trainium2 Bass kernel for nn_Downsample_Spa: sigma-conv + gaussian unfold downsample.

Math (per batch image, one NeuronCore each; batch of 8 -> 8 cores):
  xp = reflect_pad(x)                                  # [64,130,130]
  sigma[o,p] = clamp(BN(conv3x3(xp))[o,p], 1e-4)       # at stride-2 positions p only
  graw[o,p]  = exp(-0.5*d2[o]/sigma^2 - ln64) / sigma  # /64 guards fp16 range; cancels in the ratio
  out[c,p]   = sum_o graw[o,p]*xp[c,p+off(o)] / sum_o graw[o,p]

Design (v6):
 - partitions = (row-half hh, channel c) = 128; host pre-pads (reflect), fp16, and
   parity-splits columns into TWO planes (w=2j / w=2j+1); taps at w=2j+2 read
   plane0 at col+1 (still step-1). Input 2.2MB/core.
 - conv: 9 accumulating fp16 matmuls per 512-position block, block-diagonal
   weights (M=18 = both row halves). All 4 conv blocks run contiguously up front
   (dense PE stretch keeps the HAM clock-gate open); sigma f32 in PSUM.
 - g pipeline per half [18,1024]: clamp via tensor_scalar, custom-DVE fast
   reciprocal, ACT Square+Exp (per-partition scale/bias), mult -> gb bf16.
 - unfold per block: one-hot bf16 matmuls broadcast gb to the 128 partitions
   (3-tap PSUM rowgroups + Srep sum); ACT copies to one fp16 gc tile; 9 fp16
   products into two 4-slot tiles -> one 2x-mode t4 add, then t2/t1/+center,
   *1/Srep; tail offloaded to gpsimd on early blocks; fp16 out DMA per block.
 - warm-up matmuls on a memset scratch tile (no DMA dependency); inputs DMA'd
   over both hw queues (sync+scalar) so chunk 0 lands in ~2us.
"""

import os
import sys

import numpy as np

if "/opt/trn_rl_repo" not in sys.path:
    sys.path.insert(0, "/opt/trn_rl_repo")

K = 3
BN_EPS = 1e-5
SIGMA_MIN = 1e-4
GSCALE_LN = float(np.log(64.0))   # graw scaled by 1/64 (folded into exp bias)
N, C, H, W = 8, 64, 128, 128
HO = WO = 64
HH = 2
RS = 65                  # padded-row slots per partition-half
HOC = 32
NBLK = 4
BR = HOC // NBLK         # 8 output rows per block
NPOS = BR * WO           # 512
NP2 = 2 * NPOS
PL = 2                   # x col-parity planes: w=2j / w=2j+1
JW = 66                  # j slots per plane (65 used, 66 for alignment)
CR = 17                  # rows per DMA chunk tile (16 + 1 overlap)

# f32 consts tensor columns
_D2 = 0                  # -0.5*d2[o] per (hh,o)
_BC = 1                  # bn_bias - sigma_min
_LB = 2                  # exp bias: constant -ln(64) per partition
_EPS = 3                 # sigma clamp floor
_NCC = 4

_STATE = {}


def _build_consts(conv_w, bn_gamma, bn_beta, bn_mean, bn_var):
    s = (bn_gamma / np.sqrt(bn_var + BN_EPS)).astype(np.float32)
    wf = conv_w.astype(np.float32) * s[:, None, None, None]           # [9,64,3,3]
    bias = (bn_beta - bn_mean * s).astype(np.float32)

    cst = np.zeros((18, _NCC), np.float32)
    d2 = np.array([(kk // 3 - 1) ** 2 + (kk % 3 - 1) ** 2 for kk in range(9)], np.float32)
    for hh in range(HH):
        cst[hh * 9:hh * 9 + 9, _D2] = -0.5 * d2
        cst[hh * 9:hh * 9 + 9, _BC] = bias - SIGMA_MIN
        cst[hh * 9:hh * 9 + 9, _LB] = -GSCALE_LN
        cst[hh * 9:hh * 9 + 9, _EPS] = SIGMA_MIN

    # conv weights, block-diagonal per tap: win[k=hh*64+c, tap*18 + hh*9+o]
    win = np.zeros((128, 9 * 18), np.float16)
    for tap in range(9):
        i, j = tap // 3, tap % 3
        for hh in range(HH):
            win[hh * 64:hh * 64 + 64, tap * 18 + hh * 9:tap * 18 + hh * 9 + 9] = \
                wf[:, :, i, j].T.astype(np.float16)

    # one-hot / ones broadcast weights: gin[k=hh*9+o, tap*128 + hh*64+c]
    import ml_dtypes
    gin = np.zeros((18, 10 * 128), ml_dtypes.bfloat16)
    for hh in range(HH):
        gin[hh * 9:hh * 9 + 9, 9 * 128 + hh * 64:9 * 128 + hh * 64 + 64] = 1.0
        for tap in range(9):
            gin[hh * 9 + tap, tap * 128 + hh * 64:tap * 128 + hh * 64 + 64] = 1.0
    return cst, win, gin


def _build_bass(for_sim=False):
    import concourse.bass as bass
    import concourse.tile as tile
    from concourse import mybir

    f32 = mybir.dt.float32
    f16 = mybir.dt.float16
    bf16 = mybir.dt.bfloat16
    MULT = mybir.AluOpType.mult
    ADD = mybir.AluOpType.add
    MAX = mybir.AluOpType.max
    AF = mybir.ActivationFunctionType

    if for_sim:
        nc = bass.Bass("TRN2", target_bir_lowering=False, detect_race_conditions=False)
    else:
        from concourse import bacc
        nc = bacc.Bacc()
    xin = nc.dram_tensor("xin", [128, RS, PL, JW], f16, kind="ExternalInput")
    cin = nc.dram_tensor("cin", [18, _NCC], f32, kind="ExternalInput")
    win = nc.dram_tensor("win", [128, 9 * 18], f16, kind="ExternalInput")
    gin = nc.dram_tensor("gin", [18, 10 * 128], bf16, kind="ExternalInput")
    out = nc.dram_tensor("out", [128, HOC, WO], f16, kind="ExternalOutput")

    with tile.TileContext(nc) as tc:
        from contextlib import ExitStack
        with ExitStack() as ctx:
            big = ctx.enter_context(tc.tile_pool(name="big", bufs=1))
            gsb = ctx.enter_context(tc.tile_pool(name="gsb", bufs=3))
            y_p = ctx.enter_context(tc.tile_pool(name="y", bufs=3))
            ps_s = ctx.enter_context(tc.tile_pool(name="ps_s", bufs=2, space="PSUM"))
            ps_g = ctx.enter_context(tc.tile_pool(name="ps_g", bufs=2, space="PSUM"))

            # warm-up operand: memset scratch, so the PE can start before any DMA
            scr = big.tile([128, 162], f16)
            nc.vector.memset(scr[:], 0.0)

            # ---- input DMAs over both hw queues; chunk 0 first ----
            ws = big.tile([128, 9 * 18], f16)
            cs = big.tile([18, _NCC], f32)
            gs = big.tile([18, 10 * 128], bf16)
            # 4 input chunks FIFO on the sync hw queue: chunk 0 gets the full
            # queue rate and lands first; tiny consts ride the scalar queue
            xsk = []
            for b in range(NBLK):
                xs = big.tile([128, CR, PL, JW], f16, tag=f"xs{b}")
                xsk.append(xs)
            nc.scalar.dma_start(out=ws[:], in_=win[:])
            nc.sync.dma_start(out=xsk[0][:], in_=xin[:, 0:CR, :, :])
            nc.scalar.dma_start(out=cs[:], in_=cin[:])
            nc.scalar.dma_start(out=gs[:], in_=gin[:])
            nc.sync.dma_start(out=xsk[1][:], in_=xin[:, 16:16 + CR, :, :])
            nc.sync.dma_start(out=xsk[2][:], in_=xin[:, 32:32 + CR, :, :])
            nc.sync.dma_start(out=xsk[3][:], in_=xin[:, 48:48 + CR, :, :])

            def xtap(tap, blk):
                # [128, 8, 64] fp16 step-1 view for tap (i,j) in block blk
                i, j = tap // 3, tap % 3
                if j < 2:
                    return xsk[blk][:, i:i + 2 * BR - 1:2, j, 0:WO]
                return xsk[blk][:, i:i + 2 * BR - 1:2, 0, 1:WO + 1]

            def xpair(i, blk):
                # [128, 2, 8, 64] planes (w=2j, 2j+1) for tap row i
                return xsk[blk][:, i:i + 2 * BR - 1:2, 0:PL, 0:WO].transpose([0, 2, 1, 3])

            # ---- PE warm-up on scratch (~3.4us) so HAM un-throttles pre-conv ----
            wu = ps_s.tile([18, NPOS], f32, tag="sig")
            for _ in range(30):
                nc.tensor.matmul(wu[:, 0:162], scr[:, 0:18], scr[:, 0:162],
                                 start=True, stop=True)

            # ---- conv: per-block sigma [18,512] in PSUM, clamped into a
            # [18,1024] SBUF half tile ----
            sigh = []
            for h in range(2):
                sc = gsb.tile([18, NP2], f32, tag=f"sc{h}")
                for sub in range(2):
                    blk = 2 * h + sub
                    sig = ps_s.tile([18, NPOS], f32, tag="sig")
                    for tap in range(9):
                        nc.tensor.matmul(
                            sig[:],
                            ws[:, tap * 18:(tap + 1) * 18],
                            xtap(tap, blk),
                            start=(tap == 0), stop=(tap == 8),
                        )
                    # clamp on ACT, not DVE: max(v, f) == Relu(v - f) + f exactly
                    sp = gsb.tile([18, NPOS], f32, tag="sp")
                    nc.scalar.activation(out=sp[:], in_=sig[:], func=AF.Relu,
                                         bias=cs[:, _BC:_BC + 1])
                    nc.scalar.activation(out=sc[:, sub * NPOS:(sub + 1) * NPOS],
                                         in_=sp[:], func=AF.Identity,
                                         bias=cs[:, _EPS:_EPS + 1])
                sigh.append(sc)

            def g_emit(sc, lo, hi):
                npo = hi - lo
                inv = gsb.tile([18, NP2], f32, tag="inv")
                nc.vector.reciprocal_approx_fast(out=inv[:, 0:npo], in_=sc[:, lo:hi])
                qt = gsb.tile([18, NP2], f32, tag="qt")
                nc.scalar.activation(out=qt[:, 0:npo], in_=inv[:, 0:npo], func=AF.Square)
                et = gsb.tile([18, NP2], f32, tag="et")
                nc.scalar.activation(out=et[:, 0:npo], in_=qt[:, 0:npo], func=AF.Exp,
                                     scale=cs[:, _D2:_D2 + 1],
                                     bias=cs[:, _LB:_LB + 1])
                gb = gsb.tile([18, NP2], bf16, tag="gb")
                nc.vector.tensor_tensor(out=gb[:, 0:npo], in0=et[:, 0:npo],
                                        in1=inv[:, 0:npo], op=MULT)
                return gb

            def unfold_emit(blk, gbs, late):
                # gbs: [18, 512] slice (this block's g, taps on partitions)
                Srep = ps_s.tile([128, NPOS], f32, tag="sig")
                nc.tensor.matmul(Srep[:], gs[:, 9 * 128:10 * 128], gbs,
                                 start=True, stop=True)
                rr = y_p.tile([128, BR, WO], f32, tag="rr")
                nc.vector.reciprocal_approx_fast(out=rr[:], in_=Srep[:])

                gc = y_p.tile([128, 9, BR, WO], f16, tag="gc")
                for g in range(3):  # rowgroups: taps (0,1,2), (3,4,5), (6,7,8)
                    g3 = ps_g.tile([128, 3, NPOS], f32, tag="grep")
                    for k in range(3):
                        tap = 3 * g + k
                        nc.tensor.matmul(g3[:, k], gs[:, tap * 128:(tap + 1) * 128],
                                         gbs, start=True, stop=True)
                    nc.scalar.activation(out=gc[:, 3 * g:3 * g + 3], in_=g3[:],
                                         func=AF.Copy)

                # products, all fp16: ytA = [r0j0, r0j1, r0j2, r1j0],
                # ytB = [r2j0, r2j1, r2j2, r1j2], yC = center
                ytA = y_p.tile([128, 4, BR, WO], f16, tag="ytA")
                ytB = y_p.tile([128, 4, BR, WO], f16, tag="ytB")
                yC = y_p.tile([128, BR, WO], f16, tag="yC")
                nc.vector.tensor_tensor(out=ytA[:, 0:2], in0=xpair(0, blk),
                                        in1=gc[:, 0:2], op=MULT)
                nc.vector.tensor_tensor(out=ytA[:, 2], in0=xtap(2, blk),
                                        in1=gc[:, 2], op=MULT)
                nc.vector.tensor_tensor(out=ytA[:, 3], in0=xtap(3, blk),
                                        in1=gc[:, 3], op=MULT)
                nc.vector.tensor_tensor(out=yC[:], in0=xtap(4, blk),
                                        in1=gc[:, 4], op=MULT)
                nc.vector.tensor_tensor(out=ytB[:, 3], in0=xtap(5, blk),
                                        in1=gc[:, 5], op=MULT)
                nc.vector.tensor_tensor(out=ytB[:, 0:2], in0=xpair(2, blk),
                                        in1=gc[:, 6:8], op=MULT)
                nc.vector.tensor_tensor(out=ytB[:, 2], in0=xtap(8, blk),
                                        in1=gc[:, 8], op=MULT)

                # pair tree (fp16, 2x-mode contiguous tiles) + center + normalize
                t4 = y_p.tile([128, 4, BR, WO], f16, tag="t4")
                nc.vector.tensor_tensor(out=t4[:], in0=ytA[:], in1=ytB[:], op=ADD)
                eng1 = nc.vector if late else nc.gpsimd
                t2 = y_p.tile([128, 2, BR, WO], f16, tag="t2")
                nc.vector.tensor_tensor(out=t2[:], in0=t4[:, 0:2], in1=t4[:, 2:4], op=ADD)
                t1 = y_p.tile([128, BR, WO], f16, tag="t1")
                eng1.tensor_tensor(out=t1[:], in0=t2[:, 0], in1=t2[:, 1], op=ADD)
                tC = y_p.tile([128, BR, WO], f16, tag="tC")
                eng1.tensor_tensor(out=tC[:], in0=t1[:], in1=yC[:], op=ADD)
                o16 = y_p.tile([128, BR, WO], f16, tag="o16")
                eng1.tensor_tensor(out=o16[:], in0=tC[:], in1=rr[:], op=MULT)
                nc.sync.dma_start(out=out[:, BR * blk:BR * (blk + 1), :], in_=o16[:])

            gb0a = g_emit(sigh[0], 0, NPOS)        # block 0 alone: starts DVE early
            unfold_emit(0, gb0a[:, 0:NPOS], late=False)
            gb0b = g_emit(sigh[0], NPOS, NP2)
            unfold_emit(1, gb0b[:, 0:NPOS], late=False)
            gb1 = g_emit(sigh[1], 0, NP2)
            unfold_emit(2, gb1[:, 0:NPOS], late=False)
            unfold_emit(3, gb1[:, NPOS:NP2], late=True)

    if not for_sim and not nc.is_finalized():
        nc.finalize()
    return nc


def _prep_inputs(x, conv_w, bn_gamma, bn_beta, bn_mean, bn_var):
    cst, win, gin = _build_consts(conv_w, bn_gamma, bn_beta, bn_mean, bn_var)
    xp = np.pad(np.asarray(x, np.float32), ((0, 0), (0, 0), (1, 1), (1, 1)),
                mode="reflect").astype(np.float16)                    # [8,64,130,130]
    in_maps = []
    for n in range(N):
        xc = np.concatenate([xp[n, :, 0:RS, :], xp[n, :, 64:64 + RS, :]], axis=0)
        xpl = np.zeros((128, RS, PL, JW), np.float16)
        xpl[:, :, 0, 0:65] = xc[:, :, 0:130:2]
        xpl[:, :, 1, 0:65] = xc[:, :, 1:130:2]
        in_maps.append({"xin": xpl, "cin": cst, "win": win, "gin": gin})
    return in_maps


def _gather(results):
    out = np.empty((N, C, HO, WO), np.float32)
    for n in range(N):
        d = np.asarray(results[n]["out"], np.float32)
        out[n, :, 0:HOC, :] = d[0:64]
        out[n, :, HOC:, :] = d[64:128]
    return out


def _enable_axon_trace():
    """Register the NTFF profile hook that this image's antenv lacks."""
    if _STATE.get("trace_hooked"):
        return
    import types
    import antenv
    from concourse import bass_utils
    mod = types.ModuleType("antenv.axon_hooks")
    mod._hook = None
    mod.set_axon_ntff_profile_hook = lambda h: setattr(mod, "_hook", h)
    mod.get_axon_ntff_profile_hook = lambda: mod._hook
    sys.modules["antenv.axon_hooks"] = mod
    antenv.axon_hooks = mod
    from trn_agent_boot.trn_boot import _ntff_profile_via_ctypes
    mod._hook = _ntff_profile_via_ctypes("/opt/axon/libaxon_pjrt.so")
    bass_utils.upload_artifacts = lambda tmpdir: tmpdir
    _STATE["trace_hooked"] = True


def run(x, conv_w, bn_gamma, bn_beta, bn_mean, bn_var, trace=False):
    from concourse.bass_utils import run_bass_kernel_spmd
    if trace:
        _enable_axon_trace()
    if "nc" not in _STATE:
        _STATE["nc"] = _build_bass()
    in_maps = _prep_inputs(x, conv_w, bn_gamma, bn_beta, bn_mean, bn_var)
    res = run_bass_kernel_spmd(_STATE["nc"], in_maps, list(range(N)), trace=trace)
    _STATE["last"] = res
    return _gather(res.results)


def kernel(x, conv_w, bn_gamma, bn_beta, bn_mean, bn_var):
    return run(x, conv_w, bn_gamma, bn_beta, bn_mean, bn_var,
               trace=bool(int(os.environ.get("KERNEL_TRACE", "0"))))
